# revision 18
# baseline (speedup 1.0000x reference)
"""Trainium2 Bass kernel for nn_Decoder_BCD (Gumbel-Sinkhorn permutation decoder).

Strategy (8 cores):
- MLP: W1 sharded over contraction dim (K) -> AllReduce h1 (tiny);
  W2 replicated; W3 sharded over K (64 rows/core) -> AllReduce raw logits
  (2MB) -> each core indirect-gathers its own 4 batches' logit matrices.
- Everything downstream (Sinkhorn, greedy matching, (I-PLP^T)^-1 via
  nilpotent product, ancestral sampling in closed form, decode) is
  batch-parallel: 4 batches per core.
- qz closed form: with Bm = P L P^T, A = (I-Bm)^-1 = prod_k (I + Bm^(2^k)),
  qz[s] = s0 - s0[it_s]*A[:,it_s] where s0 = A @ (0.1*z_anc[s]).
"""
import os
import sys
import numpy as np
from contextlib import ExitStack

for p in ("/opt/trn_rl_repo", "/root/.axon_site/_ro/trn_rl_repo"):
    if os.path.isdir(p) and p not in sys.path:
        sys.path.append(p)

import concourse.bass as bass
import concourse.mybir as mybir
from concourse.tile import TileContext
from concourse.bass_utils import run_bass_kernel_spmd
from concourse.masks import make_identity

DIM = 128
LD = 8128
B = 32
S = 2048
HID = 512
PROJ = 256
NCORES = 8
BL = B // NCORES            # 4 batches per core
KPAD = 8192                 # W1 contraction padded
KCH = KPAD // NCORES        # 1024 rows of W1 per core
K3 = HID // NCORES          # 64 rows of W3 per core
NST = S // DIM              # 16 sample tiles
LOG_SIG = float(np.log(0.1))
C2 = float(0.5 * np.log(2.0 * np.pi) * LD)
GELU_A = 0.7978845608028654
GELU_B = 0.044715
f32 = mybir.dt.float32
i32 = mybir.dt.int32
AF = mybir.ActivationFunctionType
ALU = mybir.AluOpType
AX = mybir.AxisListType

_CACHE = {}


def _legalize_waits(nc):
    """This walrus build allows only one sync-wait per instruction; hoist
    extra waits into standalone EventSemaphore ops on the same engine."""
    n = 0
    for f in nc.m.functions:
        for bb in f.blocks:
            insts = list(bb.instructions)
            out = []
            changed = False
            for inst in insts:
                si = getattr(inst, "sync_info", None)
                waits = list(si.on_wait) if si is not None else []
                if len(waits) > 1:
                    for w in waits[:-1]:
                        n += 1
                        out.append(mybir.InstEventSemaphore(
                            name=f"LGW-{n}",
                            engine=inst.engine,
                            ins=[], outs=[],
                            sync_info=mybir.SyncInfo(on_wait=[w], on_update=[])))
                    inst.sync_info = mybir.SyncInfo(
                        on_wait=[waits[-1]], on_update=list(si.on_update))
                    changed = True
                out.append(inst)
            if changed:
                bb.instructions = out
    return n


def _build(ph=0xFF):
    nc = bass.Bass()

    # ---- per-core external inputs (host-sharded) ----
    W1c = nc.declare_dram_parameter("W1c", [KCH, HID], f32, isOutput=False)
    W2f = nc.declare_dram_parameter("W2f", [HID, HID], f32, isOutput=False)
    W3c = nc.declare_dram_parameter("W3c", [K3, DIM * DIM], f32, isOutput=False)
    Wdec = nc.declare_dram_parameter("Wdec", [DIM, PROJ], f32, isOutput=False)
    zTc = nc.declare_dram_parameter("zTc", [KCH, B], f32, isOutput=False)
    meansc = nc.declare_dram_parameter("meansc", [KCH], f32, isOutput=False)
    lstdc = nc.declare_dram_parameter("lstdc", [KCH], f32, isOutput=False)
    selc = nc.declare_dram_parameter("selc", [HID, K3], f32, isOutput=False)
    zrow = nc.declare_dram_parameter("zrow", [BL, LD], f32, isOutput=False)
    meansf = nc.declare_dram_parameter("meansf", [LD], f32, isOutput=False)
    lstdf = nc.declare_dram_parameter("lstdf", [LD], f32, isOutput=False)
    trimean = nc.declare_dram_parameter("trimean", [DIM, DIM], f32, isOutput=False)
    trilstd = nc.declare_dram_parameter("trilstd", [DIM, DIM], f32, isOutput=False)
    ztri = nc.declare_dram_parameter("ztri", [BL, DIM, DIM], f32, isOutput=False)
    gumb = nc.declare_dram_parameter("gumb", [BL, DIM, DIM], f32, isOutput=False)
    zaT = nc.declare_dram_parameter("zaT", [DIM, S], f32, isOutput=False)
    itv = nc.declare_dram_parameter("itv", [S], i32, isOutput=False)
    pidx = nc.declare_dram_parameter("pidx", [DIM, BL], i32, isOutput=False)

    # ---- per-core outputs (shards) ----
    o_P = nc.declare_dram_parameter("o_P", [BL, DIM, DIM], f32, isOutput=True)
    o_PL = nc.declare_dram_parameter("o_PL", [BL, DIM, DIM], f32, isOutput=True)
    o_L = nc.declare_dram_parameter("o_L", [BL, DIM, DIM], f32, isOutput=True)
    o_LN = nc.declare_dram_parameter("o_LN", [BL, DIM], f32, isOutput=True)
    o_W = nc.declare_dram_parameter("o_W", [BL, DIM, DIM], f32, isOutput=True)
    o_qz = nc.declare_dram_parameter("o_qz", [BL, S, DIM], f32, isOutput=True)
    o_FL = nc.declare_dram_parameter("o_FL", [BL, LD + 1], f32, isOutput=True)
    o_flp = nc.declare_dram_parameter("o_flp", [BL], f32, isOutput=True)
    o_X = nc.declare_dram_parameter("o_X", [BL, S, PROJ], f32, isOutput=True)

    # ---- internal DRAM ----
    h1p = nc.dram_tensor("h1p", [B, HID], f32)
    h1r = nc.dram_tensor("h1r", [B, HID], f32, addr_space="Shared")
    ppart = nc.dram_tensor("ppart", [B, DIM * DIM], f32)
    pgfull = nc.dram_tensor("pgfull", [B, DIM * DIM], f32, addr_space="Shared")
    lga = nc.dram_tensor("lga", [BL, DIM, DIM], f32)
    groups = [list(range(NCORES))]

    with TileContext(nc) as tc, ExitStack() as ctx:
        per = ctx.enter_context(tc.tile_pool(name="per", bufs=1))
        wk = ctx.enter_context(tc.tile_pool(name="wk", bufs=3))
        ps = ctx.enter_context(tc.tile_pool(name="ps", bufs=2, space="PSUM"))
        gr = ctx.enter_context(tc.tile_pool(name="gr", bufs=2))

        ident = per.tile([DIM, DIM], f32, tag="ident")
        make_identity(nc, ident)
        ones_col = per.tile([DIM, 1], f32, tag="ones_col")
        nc.vector.memset(ones_col, 1.0)
        ones_row = per.tile([1, DIM], f32, tag="ones_row")
        nc.vector.memset(ones_row, 1.0)

        # ============ MLP phase ============
        # l^T chunk: (128, 8, 32) = means + exp(10*tanh(raw/10)) * z
        sb_zT = per.tile([DIM, KCH // DIM, B], f32, tag="sb_zT")
        zTc_r = zTc.rearrange("(j p) b -> p j b", p=DIM)
        m8r = per.tile([DIM, KCH // DIM], f32, tag="m8r")
        nc.gpsimd.dma_start(out=m8r, in_=meansc.rearrange("(j p) -> p j", p=DIM))
        m8 = per.tile([DIM, KCH // DIM], f32, tag="m8")
        nc.scalar.activation(out=m8, in_=m8r, func=AF.Identity)
        s8 = per.tile([DIM, KCH // DIM], f32, tag="s8")
        nc.gpsimd.dma_start(out=s8, in_=lstdc.rearrange("(j p) -> p j", p=DIM))
        w1s = per.tile([DIM, KCH // DIM, HID], f32, tag="w1s")
        w1_r = W1c.rearrange("(j p) n -> p j n", p=DIM)
        for j in range(KCH // DIM):
            nc.gpsimd.dma_start(out=sb_zT[:, j], in_=zTc_r[:, j])
            nc.gpsimd.dma_start(out=w1s[:, j], in_=w1_r[:, j])

        std8 = per.tile([DIM, KCH // DIM], f32, tag="std8")
        for j in range(KCH // DIM):
            nc.scalar.activation(out=std8[:, j : j + 1], in_=s8[:, j : j + 1],
                                 func=AF.Tanh, scale=0.1)
            nc.scalar.activation(out=std8[:, j : j + 1], in_=std8[:, j : j + 1],
                                 func=AF.Exp, scale=10.0)
            nc.scalar.activation(out=sb_zT[:, j], in_=sb_zT[:, j],
                                 func=AF.Identity,
                                 scale=std8[:, j : j + 1],
                                 bias=m8[:, j : j + 1])
        lT = sb_zT
        h1_ps = ps.tile([B, HID], f32, tag="mm512")
        for j in range(KCH // DIM):
            nc.tensor.matmul(h1_ps, lT[:, j], w1s[:, j],
                             start=(j == 0), stop=(j == KCH // DIM - 1))
        h1sb = per.tile([B, HID], f32, tag="h1sb")
        nc.vector.tensor_copy(out=h1sb, in_=h1_ps)
        nc.sync.dma_start(out=h1p[:], in_=h1sb)
        nc.gpsimd.collective_compute(
            "AllReduce", ALU.add, replica_groups=groups, ins=[h1p[:]], outs=[h1r[:]])
        hsb = per.tile([B, HID], f32, tag="hsb")
        nc.gpsimd.dma_start(out=hsb, in_=h1r[:])

        def gelu(dst, src, tmp_tag):
            x2 = wk.tile([B, HID], f32, tag=tmp_tag + "a")
            nc.vector.tensor_mul(out=x2, in0=src, in1=src)
            nc.vector.tensor_mul(out=x2, in0=x2, in1=src)
            nc.vector.scalar_tensor_tensor(out=x2, in0=x2, scalar=GELU_B, in1=src,
                                           op0=ALU.mult, op1=ALU.add)
            nc.scalar.activation(out=x2, in_=x2, func=AF.Tanh, scale=GELU_A)
            nc.vector.scalar_tensor_tensor(out=x2, in0=x2, scalar=1.0, in1=src,
                                           op0=ALU.add, op1=ALU.mult)
            nc.vector.tensor_scalar_mul(dst, x2, 0.5)

        hg = per.tile([B, HID], f32, tag="hg")
        gelu(hg, hsb, "g1")
        # h1^T : (128, 4, 32)
        h1T = per.tile([DIM, HID // DIM, B], f32, tag="h1T")
        for t in range(HID // DIM):
            tp = ps.tile([DIM, B], f32, tag="misc")
            nc.tensor.transpose(tp, hg[:, t * DIM : (t + 1) * DIM], ident[:B, :B])
            nc.vector.tensor_copy(out=h1T[:, t], in_=tp)
        w2s = per.tile([DIM, HID // DIM, HID], f32, tag="w2s")
        w2_r = W2f.rearrange("(j p) n -> p j n", p=DIM)
        for t in range(HID // DIM):
            nc.gpsimd.dma_start(out=w2s[:, t], in_=w2_r[:, t])
        h2_ps = ps.tile([B, HID], f32, tag="mm512")
        for t in range(HID // DIM):
            nc.tensor.matmul(h2_ps, h1T[:, t], w2s[:, t],
                             start=(t == 0), stop=(t == HID // DIM - 1))
        h2g = per.tile([B, HID], f32, tag="h2g")
        nc.vector.tensor_copy(out=h2g, in_=h2_ps)
        gelu(h2g, h2g, "g2")
        h2T = per.tile([DIM, HID // DIM, B], f32, tag="h2T")
        for t in range(HID // DIM):
            tp = ps.tile([DIM, B], f32, tag="misc")
            nc.tensor.transpose(tp, h2g[:, t * DIM : (t + 1) * DIM], ident[:B, :B])
            nc.vector.tensor_copy(out=h2T[:, t], in_=tp)

        # select this core's 64 rows of h2^T via host one-hot selector
        sels = per.tile([DIM, HID // DIM, K3], f32, tag="sels")
        sel_r = selc.rearrange("(j p) n -> p j n", p=DIM)
        for t in range(HID // DIM):
            nc.gpsimd.dma_start(out=sels[:, t], in_=sel_r[:, t])
        h2c_ps = ps.tile([K3, B], f32, tag="misc")
        for t in range(HID // DIM):
            nc.tensor.matmul(h2c_ps, sels[:, t], h2T[:, t],
                             start=(t == 0), stop=(t == HID // DIM - 1))
        h2c = per.tile([K3, B], f32, tag="h2c")
        nc.vector.tensor_copy(out=h2c, in_=h2c_ps)

        for ns in range(DIM * DIM // HID):  # 32 chunks of 512
            w3t = wk.tile([K3, HID], f32, tag="w3t")
            nc.gpsimd.dma_start(out=w3t, in_=W3c[:, ns * HID : (ns + 1) * HID])
            pp = ps.tile([B, HID], f32, tag="mm512")
            nc.tensor.matmul(pp, h2c, w3t, start=True, stop=True)
            pout = wk.tile([B, HID], f32, tag="pout")
            nc.vector.tensor_copy(out=pout, in_=pp)
            nc.sync.dma_start(out=ppart[:, ns * HID : (ns + 1) * HID], in_=pout)
        nc.gpsimd.collective_compute(
            "AllReduce", ALU.add, replica_groups=groups,
            ins=[ppart[:]], outs=[pgfull[:]])

        # gather this core's 4 batches' raw logit matrices
        pidx_sb = per.tile([DIM, BL], i32, tag="pidx_sb")
        nc.gpsimd.dma_start(out=pidx_sb, in_=pidx[:])
        xt = per.tile([DIM, BL, DIM], f32, tag="xt")
        pg_view = pgfull[:].rearrange("b (i j) -> (b i) j", j=DIM)
        for lb in range(BL):
            nc.gpsimd.indirect_dma_start(
                out=xt[:, lb], out_offset=None, in_=pg_view,
                in_offset=bass.IndirectOffsetOnAxis(ap=pidx_sb[:, lb : lb + 1], axis=0))

        # P_logits = 10*tanh(raw/10); sinkhorn x0 = P_logits + gumbel
        nc.scalar.activation(out=xt, in_=xt, func=AF.Tanh, scale=0.1)
        rs = per.tile([DIM, BL], f32, tag="rs")
        et = per.tile([DIM, BL * DIM], f32, tag="et")
        plx = et[:].rearrange("p (b j) -> p b j", b=BL)
        nc.vector.tensor_scalar_mul(plx, xt, 10.0)
        nc.sync.dma_start(
            out=bass.AP(tensor=o_PL[:].tensor, offset=0,
                        ap=[[DIM, DIM], [DIM * DIM, BL], [1, DIM]]),
            in_=plx)

        gum_sb = per.tile([DIM, BL, DIM], f32, tag="gum_sb")
        gum_r = gumb.rearrange("b p j -> p b j")
        for lb in range(BL):
            nc.gpsimd.dma_start(out=gum_sb[:, lb], in_=gum_r[:, lb])
        nc.vector.tensor_scalar(out=gum_sb, in0=gum_sb, scalar1=1e-6,
                                scalar2=1.0 - 1e-6, op0=ALU.max, op1=ALU.min)
        nc.scalar.activation(out=gum_sb, in_=gum_sb, func=AF.Ln)
        nc.scalar.activation(out=gum_sb, in_=gum_sb, func=AF.Ln, scale=-1.0)
        # xt = 10*tanh - ln(-ln u)
        nc.vector.scalar_tensor_tensor(out=xt, in0=xt, scalar=10.0, in1=gum_sb,
                                       op0=ALU.mult, op1=ALU.subtract)

        # ============ Sinkhorn (20 iters, no max-subtraction) ============
        xt_flat = xt.rearrange("p b j -> p (b j)")
        for _ in range(20):
            for lb in range(BL):
                nc.scalar.activation(out=et[:, lb * DIM : (lb + 1) * DIM],
                                     in_=xt[:, lb], func=AF.Exp,
                                     accum_out=rs[:, lb : lb + 1])
            nc.scalar.activation(out=rs, in_=rs, func=AF.Ln)
            for lb in range(BL):
                nc.vector.tensor_scalar_sub(xt[:, lb], xt[:, lb], rs[:, lb : lb + 1])
            nc.scalar.activation(out=et, in_=xt_flat, func=AF.Exp)
            cs_ps = ps.tile([1, BL * DIM], f32, tag="mm512")
            nc.tensor.matmul(cs_ps, ones_col, et, start=True, stop=True)
            lc = wk.tile([1, BL * DIM], f32, tag="lc")
            nc.scalar.activation(out=lc, in_=cs_ps, func=AF.Ln)
            bc_ps = ps.tile([DIM, BL * DIM], f32, tag="mm512")
            nc.tensor.matmul(bc_ps, ones_row, lc, start=True, stop=True)
            nc.vector.tensor_sub(xt_flat, xt_flat, bc_ps)

        # log_alpha -> DRAM in [b,i,j] -> back as (4, 16384)
        nc.sync.dma_start(
            out=bass.AP(tensor=lga[:].tensor, offset=0,
                        ap=[[DIM, DIM], [DIM * DIM, BL], [1, DIM]]),
            in_=xt)
        xg = per.tile([BL, DIM * DIM], f32, tag="xg")
        lga_f = lga[:].rearrange("b p j -> b (p j)")
        for q in range(4):
            qs = q * (DIM * DIM // 4)
            nc.gpsimd.dma_start(out=xg[:, qs : qs + DIM * DIM // 4],
                              in_=lga_f[:, qs : qs + DIM * DIM // 4])

        # ============ greedy matching (sequential, 128 steps) ============
        iota_i = per.tile([DIM, DIM], i32, tag="iota_i")
        nc.gpsimd.iota(iota_i, pattern=[[1, DIM]], base=0, channel_multiplier=0)
        iota_f = per.tile([DIM, DIM], f32, tag="iota_f")
        nc.vector.tensor_copy(out=iota_f, in_=iota_i)
        used = per.tile([BL, DIM], f32, tag="used")
        nc.vector.memset(used, 0.0)
        for i in range(DIM):
            sl = slice(i * DIM, (i + 1) * DIM)
            mrow = gr.tile([BL, DIM], f32, tag="mrow")
            nc.vector.tensor_add(out=mrow, in0=xg[:, sl], in1=used)
            mx = gr.tile([BL, 8], f32, tag="mx")
            nc.vector.max(mx, mrow)
            ix = gr.tile([BL, 8], mybir.dt.uint32, tag="ix")
            nc.vector.max_index(ix, mx, mrow)
            cf = gr.tile([BL, 1], f32, tag="cf")
            nc.vector.tensor_copy(out=cf, in_=ix[:, :1])
            prow = gr.tile([BL, DIM], f32, tag="prow")
            nc.vector.tensor_scalar(out=prow, in0=iota_f[:BL], scalar1=cf,
                                    scalar2=None, op0=ALU.is_equal)
            nc.vector.scalar_tensor_tensor(out=used, in0=prow, scalar=-1e30,
                                           in1=used, op0=ALU.mult, op1=ALU.add)
            nc.sync.dma_start(
                out=bass.AP(tensor=o_P[:].tensor, offset=i * DIM,
                            ap=[[DIM * DIM, BL], [1, DIM]]),
                in_=prow)

        # ============ per-batch: L, W=(PLP^T)^T, A=(I-PLP^T)^-1, AW ============
        tri_t = per.tile([DIM, DIM], f32, tag="tri_t")
        nc.scalar.activation(out=tri_t, in_=trilstd_sb(nc, per, trilstd),
                             func=AF.Tanh, scale=0.1)
        std_tri = per.tile([DIM, DIM], f32, tag="std_tri")
        nc.scalar.activation(out=std_tri, in_=tri_t, func=AF.Exp, scale=10.0)
        mean_tri = per.tile([DIM, DIM], f32, tag="mean_tri")
        nc.gpsimd.dma_start(out=mean_tri, in_=trimean[:])
        ztri_sb = per.tile([DIM, BL, DIM], f32, tag="ztri_sb")
        ztri_r = ztri.rearrange("b p j -> p b j")
        for lb in range(BL):
            nc.gpsimd.dma_start(out=ztri_sb[:, lb], in_=ztri_r[:, lb])

        wdec_sb = per.tile([DIM, PROJ], f32, tag="wdec_sb")
        nc.gpsimd.dma_start(out=wdec_sb, in_=Wdec[:])
        ATs, AWs = [], []
        for b in range(BL):
            Lb = per.tile([DIM, DIM], f32, tag=f"Lb{b}")
            nc.vector.tensor_mul(out=Lb, in0=ztri_sb[:, b], in1=std_tri)
            nc.vector.tensor_add(out=Lb, in0=Lb, in1=mean_tri)
            nc.sync.dma_start(out=o_L[b], in_=Lb)

            Pt = wk.tile([DIM, DIM], f32, tag="Pt")
            nc.gpsimd.dma_start(out=Pt, in_=o_P[b])
            PT_ps = ps.tile([DIM, DIM], f32, tag="mm128")
            nc.tensor.transpose(PT_ps, Pt, ident)
            PTs = wk.tile([DIM, DIM], f32, tag="PTs")
            nc.vector.tensor_copy(out=PTs, in_=PT_ps)

            Y_ps = ps.tile([DIM, DIM], f32, tag="mm128")
            nc.tensor.matmul(Y_ps, PTs, Lb, start=True, stop=True)  # P L
            Ys = wk.tile([DIM, DIM], f32, tag="Ys")
            nc.vector.tensor_copy(out=Ys, in_=Y_ps)
            YT_ps = ps.tile([DIM, DIM], f32, tag="mm128")
            nc.tensor.transpose(YT_ps, Ys, ident)
            YTs = wk.tile([DIM, DIM], f32, tag="YTs")
            nc.vector.tensor_copy(out=YTs, in_=YT_ps)
            G2_ps = ps.tile([DIM, DIM], f32, tag="mm128")
            nc.tensor.matmul(G2_ps, PTs, YTs, start=True, stop=True)  # (PLP^T)^T
            G2s = wk.tile([DIM, DIM], f32, tag="G2s")
            nc.vector.tensor_copy(out=G2s, in_=G2_ps)
            nc.sync.dma_start(out=o_W[b], in_=G2s)

            G1_ps = ps.tile([DIM, DIM], f32, tag="mm128")
            nc.tensor.transpose(G1_ps, G2s, ident)
            Cs = wk.tile([DIM, DIM], f32, tag="Cs")
            nc.vector.tensor_copy(out=Cs, in_=G1_ps)  # C0 = PLP^T
            Ds = G2s                                   # D0 = C0^T
            Ns = wk.tile([DIM, DIM], f32, tag="Ns")
            nc.vector.tensor_add(out=Ns, in0=Ds, in1=ident)  # N0 = I + D0
            for _k in range(6):
                Cp = ps.tile([DIM, DIM], f32, tag="mm128")
                nc.tensor.matmul(Cp, Ds, Cs, start=True, stop=True)  # C^2
                Dp = ps.tile([DIM, DIM], f32, tag="mm128")
                nc.tensor.matmul(Dp, Cs, Ds, start=True, stop=True)  # D^2
                Cn = wk.tile([DIM, DIM], f32, tag="Cs")
                nc.vector.tensor_copy(out=Cn, in_=Cp)
                Dn = wk.tile([DIM, DIM], f32, tag="Ds")
                nc.vector.tensor_copy(out=Dn, in_=Dp)
                Np = ps.tile([DIM, DIM], f32, tag="mm128")
                nc.tensor.matmul(Np, Cn, Ns, start=True, stop=True)  # D_k N
                N2 = wk.tile([DIM, DIM], f32, tag="Ns")
                nc.vector.tensor_add(out=N2, in0=Ns, in1=Np)
                Cs, Ds, Ns = Cn, Dn, N2
            AT = per.tile([DIM, DIM], f32, tag=f"AT{b}")
            nc.vector.tensor_copy(out=AT, in_=Ns)  # A^T
            A_ps = ps.tile([DIM, DIM], f32, tag="mm128")
            nc.tensor.transpose(A_ps, AT, ident)
            As = wk.tile([DIM, DIM], f32, tag="As")
            nc.vector.tensor_copy(out=As, in_=A_ps)
            AW_ps = ps.tile([DIM, PROJ], f32, tag="mm256")
            nc.tensor.matmul(AW_ps, As, wdec_sb, start=True, stop=True)  # A^T Wdec
            AWt = per.tile([DIM, PROJ], f32, tag=f"AW{b}")
            nc.vector.tensor_copy(out=AWt, in_=AW_ps)
            ATs.append(AT)
            AWs.append(AWt)

        # ============ decode: qz and X_recons ============
        zaTs = per.tile([DIM, S], f32, tag="zaTs")
        for q in range(4):
            sl_q = slice(q * HID, (q + 1) * HID)
            nc.gpsimd.dma_start(out=zaTs[:, sl_q], in_=zaT[:, sl_q])
            nc.vector.tensor_scalar_mul(zaTs[:, sl_q], zaTs[:, sl_q], 0.1)
        it_sb = per.tile([DIM, NST], i32, tag="it_sb")
        nc.gpsimd.dma_start(out=it_sb, in_=itv.rearrange("(st p) -> p st", p=DIM))
        it_f = per.tile([DIM, NST], f32, tag="it_f")
        nc.vector.tensor_copy(out=it_f, in_=it_sb)

        ohs, ohTs = [], []
        for st in range(NST):
            oh = per.tile([DIM, DIM], f32, tag=f"oh{st}")
            nc.vector.tensor_scalar(out=oh, in0=iota_f, scalar1=it_f[:, st : st + 1],
                                    scalar2=None, op0=ALU.is_equal)
            ohT_ps = ps.tile([DIM, DIM], f32, tag="mm128")
            nc.tensor.transpose(ohT_ps, oh, ident)
            ohT = per.tile([DIM, DIM], f32, tag=f"ohT{st}")
            nc.vector.tensor_copy(out=ohT, in_=ohT_ps)
            ohs.append(oh)
            ohTs.append(ohT)

        for b in range(BL):
            for st in range(NST):
                zsl = zaTs[:, st * DIM : (st + 1) * DIM]
                Y0 = ps.tile([DIM, DIM], f32, tag="mm512")
                nc.tensor.matmul(Y0, zsl, ATs[b], start=True, stop=True)
                gm = wk.tile([DIM, DIM], f32, tag="gm")
                nc.vector.tensor_mul(out=gm, in0=Y0, in1=ohs[st])
                gs = wk.tile([DIM, 1], f32, tag="gs")
                nc.vector.reduce_sum(gs, gm, axis=AX.X)
                AG = ps.tile([DIM, DIM], f32, tag="mm128")
                nc.tensor.matmul(AG, ohTs[st], ATs[b], start=True, stop=True)
                cor = wk.tile([DIM, DIM], f32, tag="cor")
                nc.vector.tensor_scalar_mul(cor, AG, gs)
                qzt = wk.tile([DIM, DIM], f32, tag="qzt")
                nc.vector.tensor_sub(out=qzt, in0=Y0, in1=cor)
                nc.sync.dma_start(out=o_qz[b, st * DIM : (st + 1) * DIM], in_=qzt)

                X0 = ps.tile([DIM, PROJ], f32, tag="mm256")
                nc.tensor.matmul(X0, zsl, AWs[b], start=True, stop=True)
                XG = ps.tile([DIM, PROJ], f32, tag="misc")
                nc.tensor.matmul(XG, ohTs[st], AWs[b], start=True, stop=True)
                cor2 = wk.tile([DIM, PROJ], f32, tag="cor2")
                nc.vector.tensor_scalar_mul(cor2, XG, gs)
                Xt = wk.tile([DIM, PROJ], f32, tag="Xt")
                nc.vector.tensor_sub(out=Xt, in0=X0, in1=cor2)
                nc.sync.dma_start(out=o_X[b, st * DIM : (st + 1) * DIM], in_=Xt)

        # ============ full_l rows, log-prob, log_noises ============
        zr_sb = per.tile([LD // 64, BL, 64], f32, tag="zr_sb")
        zr_r = zrow.rearrange("b (p j) -> p b j", j=64)
        for lb in range(BL):
            nc.gpsimd.dma_start(out=zr_sb[:, lb], in_=zr_r[:, lb])
        mr = per.tile([LD // 64, 64], f32, tag="mr")
        nc.gpsimd.dma_start(out=mr, in_=meansf.rearrange("(p j) -> p j", j=64))
        sr = per.tile([LD // 64, 64], f32, tag="sr")
        nc.gpsimd.dma_start(out=sr, in_=lstdf.rearrange("(p j) -> p j", j=64))
        tr = per.tile([LD // 64, 64], f32, tag="tr")
        nc.scalar.activation(out=tr, in_=sr, func=AF.Tanh, scale=0.1)
        stdr = per.tile([LD // 64, 64], f32, tag="stdr")
        nc.scalar.activation(out=stdr, in_=tr, func=AF.Exp, scale=10.0)
        flb = per.tile([LD // 64, BL, 64], f32, tag="flb")
        for b in range(BL):
            nc.vector.tensor_mul(out=flb[:, b], in0=zr_sb[:, b], in1=stdr)
            nc.vector.tensor_add(out=flb[:, b], in0=flb[:, b], in1=mr)
        nc.sync.dma_start(
            out=bass.AP(tensor=o_FL[:].tensor, offset=0,
                        ap=[[64, LD // 64], [LD + 1, BL], [1, 64]]),
            in_=flb)
        lgt = per.tile([1, BL], f32, tag="lgt")
        nc.vector.memset(lgt, LOG_SIG)
        nc.sync.dma_start(
            out=bass.AP(tensor=o_FL[:].tensor, offset=LD, ap=[[0, 1], [LD + 1, BL]]),
            in_=lgt)

        sq = per.tile([LD // 64, BL, 64], f32, tag="sq")
        nc.vector.tensor_mul(out=sq, in0=zr_sb, in1=zr_sb)
        rs2 = per.tile([LD // 64, BL], f32, tag="rs2")
        nc.vector.reduce_sum(rs2, sq, axis=AX.X)
        zz_ps = ps.tile([BL, 1], f32, tag="misc")
        nc.tensor.matmul(zz_ps, rs2, ones_col[: LD // 64], start=True, stop=True)
        # sum over log_stds (triangle): 10 * sum(tri_t)
        ts1 = per.tile([DIM, 1], f32, tag="ts1")
        nc.vector.reduce_sum(ts1, tri_t, axis=AX.X)
        ts2_ps = ps.tile([1, 1], f32, tag="misc")
        nc.tensor.matmul(ts2_ps, ts1, ones_col, start=True, stop=True)
        c2t = per.tile([1, 1], f32, tag="c2t")
        nc.vector.memset(c2t, C2)
        u1 = per.tile([1, 1], f32, tag="u1")
        nc.vector.scalar_tensor_tensor(out=u1, in0=ts2_ps, scalar=10.0, in1=c2t,
                                       op0=ALU.mult, op1=ALU.add)
        ones4 = per.tile([1, BL], f32, tag="ones4")
        nc.vector.memset(ones4, 1.0)
        ub_ps = ps.tile([BL, 1], f32, tag="misc")
        nc.tensor.matmul(ub_ps, ones4, u1, start=True, stop=True)
        ub_sb = per.tile([BL, 1], f32, tag="ub_sb")
        nc.vector.tensor_copy(out=ub_sb, in_=ub_ps)
        flpv = per.tile([BL, 1], f32, tag="flpv")
        nc.vector.scalar_tensor_tensor(out=flpv, in0=zz_ps, scalar=-0.5, in1=ub_sb,
                                       op0=ALU.mult, op1=ALU.subtract)
        nc.sync.dma_start(
            out=bass.AP(tensor=o_flp[:].tensor, offset=0, ap=[[1, BL], [0, 1]]),
            in_=flpv)

        lnt = per.tile([BL, DIM], f32, tag="lnt")
        nc.vector.memset(lnt, LOG_SIG)
        nc.sync.dma_start(out=o_LN[:], in_=lnt)

    _legalize_waits(nc)
    return nc


def trilstd_sb(nc, per, trilstd):
    t = per.tile([DIM, DIM], f32, tag="trilstd_sb")
    nc.gpsimd.dma_start(out=t, in_=trilstd[:])
    return t


def _shard_inputs(inputs):
    f = np.float32
    Lp = np.ascontiguousarray(inputs["L_params"], f)
    z_l = np.ascontiguousarray(inputs["z_l"], f)
    gum = np.ascontiguousarray(inputs["gumbel_u"], f)
    z_anc = np.ascontiguousarray(inputs["z_anc"], f)
    W1 = np.ascontiguousarray(inputs["W1"], f)
    W2 = np.ascontiguousarray(inputs["W2"], f)
    W3 = np.ascontiguousarray(inputs["W3"], f)
    Wd = np.ascontiguousarray(inputs["W_dec"], f)
    itv = np.ascontiguousarray(inputs["interv_targets"]).astype(np.int32)

    means = Lp[:LD]
    lstdr = Lp[LD:]
    rows, cols = np.triu_indices(DIM, 1)
    trimean = np.zeros((DIM, DIM), f); trimean[cols, rows] = means
    trilstd = np.zeros((DIM, DIM), f); trilstd[cols, rows] = lstdr
    ztri = np.zeros((B, DIM, DIM), f); ztri[:, cols, rows] = z_l

    zT_pad = np.zeros((KPAD, B), f); zT_pad[:LD] = z_l.T
    means_pad = np.zeros(KPAD, f); means_pad[:LD] = means
    lstd_pad = np.zeros(KPAD, f); lstd_pad[:LD] = lstdr
    W1_pad = np.zeros((KPAD, HID), f); W1_pad[:LD] = W1[:LD]
    # row 8128 of W1 pairs with constant log_sig input
    W1_pad[LD] = W1[LD]
    means_pad[LD] = LOG_SIG
    zaT = np.ascontiguousarray(z_anc.T, f)

    in_maps = []
    for c in range(NCORES):
        sel = np.zeros((HID, K3), f)
        sel[np.arange(K3) + K3 * c, np.arange(K3)] = 1.0
        pidx_arr = (np.arange(DIM)[:, None] + DIM * (BL * c + np.arange(BL))[None, :])
        m = {
            "W1c": W1_pad[KCH * c : KCH * (c + 1)],
            "W2f": W2,
            "W3c": np.ascontiguousarray(W3[K3 * c : K3 * (c + 1)]),
            "Wdec": Wd,
            "zTc": zT_pad[KCH * c : KCH * (c + 1)],
            "meansc": means_pad[KCH * c : KCH * (c + 1)],
            "lstdc": lstd_pad[KCH * c : KCH * (c + 1)],
            "selc": sel,
            "zrow": z_l[BL * c : BL * (c + 1)],
            "meansf": means,
            "lstdf": lstdr,
            "trimean": trimean,
            "trilstd": trilstd,
            "ztri": ztri[BL * c : BL * (c + 1)],
            "gumb": gum[BL * c : BL * (c + 1)],
            "zaT": zaT,
            "itv": itv,
            "pidx": np.ascontiguousarray(pidx_arr, np.int32),
        }
        in_maps.append({k: np.ascontiguousarray(v) for k, v in m.items()})
    return in_maps


def _run(inputs, trace=False):
    if "nc" not in _CACHE:
        _CACHE["nc"] = _build()
    nc = _CACHE["nc"]
    in_maps = _shard_inputs(inputs)
    out = run_bass_kernel_spmd(nc, in_maps, list(range(NCORES)), trace=trace)
    res = out.results
    cat = lambda k: np.concatenate([res[c][k] for c in range(NCORES)], axis=0)
    outs = (cat("o_P"), cat("o_PL"), cat("o_L"), cat("o_LN"), cat("o_W"),
            cat("o_qz"), cat("o_FL"), cat("o_flp"), cat("o_X"))
    return outs, out


def kernel(**inputs):
    outs, _ = _run(inputs, trace=False)
    return outs
